# revision 5
# baseline (speedup 1.0000x reference)
"""Trainium2 Bass kernel for nn_BioNet: 120-step recurrent GEMM
    X_{t+1} = mml(W @ X_t + X_full.T + bias),  X_0 = 0
on 8 NeuronCores.

Strategy (tensor-parallel row sharding):
  - Core c owns output rows R_c = [c*512, (c+1)*512) of the state X (4096 x 512).
  - W row-block (512 x 4096) lives in SBUF as bf16 lhsT tiles for the whole kernel.
  - Each step: local GEMM (bf16, fp32 PSUM accumulation) over the full gathered X,
    the bias matrix X_bias = X_full.T + bias is added inside the PSUM accumulation
    group via an fp32 identity matmul, then the mml nonlinearity:
        mml(z) = min(max(z, leak*z), 1 - 0.25/max(z, 0.5))
    computed with DVE tensor ops + reciprocal_approx_fast + one ACT op.
  - The fresh 512-row block is AllGathered (bf16) in 4 chunks of 128 rows; chunk
    DMAs land in the double-buffered X slab for the next step.  Per M-tile the
    K-loop consumes gather-group g=3 (the last to arrive) last, hiding the
    collective latency under the matmuls of groups 0-2.

Numerics: pure bf16 W/X with fp32 accumulation gives rel-L2 ~3e-4 vs the fp32
reference (measured in numpy emulation; the fixed-point iteration contracts
per-step quantization noise away).
"""
import numpy as np
import ml_dtypes

import concourse.mybir as mybir
import concourse.tile as tile
from concourse import bacc
from concourse.bass_utils import run_bass_kernel_spmd

BF16NP = ml_dtypes.bfloat16
F32 = mybir.dt.float32
BF = mybir.dt.bfloat16

LEAK = 0.01
NSTEPS = 120
NCORES = 8


def build_nc(nn=4096, nb=512, ncores=NCORES, nsteps=NSTEPS, debug=False):
    """Build the SPMD Bass graph (same program for every core)."""
    R = nn // ncores          # output rows per core
    MT = R // 128             # M tiles per core
    KT = nn // 128            # K tiles (full X row blocks)
    assert R % 128 == 0 and nn % 128 == 0

    nc = bacc.Bacc("TRN2", target_bir_lowering=False, debug=debug,
                   num_devices=ncores)

    wT_dram = nc.dram_tensor("wT", [nn, R], BF, kind="ExternalInput")
    xb_dram = nc.dram_tensor("xb", [R, nb], F32, kind="ExternalInput")
    eye_dram = nc.dram_tensor("eye", [128, 128], F32, kind="ExternalInput")
    out_dram = nc.dram_tensor("out", [R, nb], F32, kind="ExternalOutput")

    rg = [list(range(ncores))]

    with tile.TileContext(nc) as tc:
        with (
            tc.tile_pool(name="const", bufs=1) as cpool,
            tc.tile_pool(name="x", bufs=2) as xpool,
            tc.tile_pool(name="eltw", bufs=3) as epool,
            tc.tile_pool(name="ps", bufs=6, space="PSUM") as pspool,
            tc.tile_pool(name="dram", bufs=4, space="DRAM") as dpool,
        ):
            # --- resident constants -----------------------------------------
            wT = cpool.tile([128, KT, R], BF, tag="wT")
            for k in range(KT):
                nc.sync.dma_start(out=wT[:, k], in_=wT_dram[k * 128:(k + 1) * 128, :])
            xb_sb = cpool.tile([128, MT, nb], F32, tag="xb")
            for m in range(MT):
                nc.sync.dma_start(out=xb_sb[:, m], in_=xb_dram[m * 128:(m + 1) * 128, :])
            eye = cpool.tile([128, 128], F32, tag="eye")
            nc.sync.dma_start(out=eye[:], in_=eye_dram[:, :])

            # X slab layout: [128, g(MT), r(ncores), nb] so each gather group g
            # is contiguous along free dim; k-tile (r*MT + g) lives at [:, g, r].
            x_cur = None

            def epilogue(psum, m, s, x_next):
                last = (s == nsteps - 1)
                z = epool.tile([128, nb], F32, tag="z")
                u = epool.tile([128, nb], F32, tag="u")
                rr = epool.tile([128, nb], F32, tag="rr")
                v = epool.tile([128, nb], F32, tag="v")
                ll = epool.tile([128, nb], F32, tag="ll")
                # PSUM is read exactly once (walrus allows only one PSUM input per op)
                nc.scalar.activation(z[:], psum[:], mybir.ActivationFunctionType.Copy)
                nc.vector.tensor_scalar_max(u[:], z[:], 0.5)
                nc.vector.reciprocal_approx_fast(rr[:], u[:])
                nc.scalar.activation(v[:], rr[:], mybir.ActivationFunctionType.Copy,
                                     bias=1.0, scale=-0.25)
                nc.vector.scalar_tensor_tensor(ll[:], z[:], LEAK, z[:],
                                               op0=mybir.AluOpType.mult,
                                               op1=mybir.AluOpType.max)
                if last:
                    of = epool.tile([128, nb], F32, tag="of")
                    nc.vector.tensor_tensor(of[:], ll[:], v[:], op=mybir.AluOpType.min)
                    nc.sync.dma_start(out=out_dram[m * 128:(m + 1) * 128, :], in_=of[:])
                else:
                    o = epool.tile([128, nb], BF, tag="o")
                    nc.vector.tensor_tensor(o[:], ll[:], v[:], op=mybir.AluOpType.min)
                    ag_in = dpool.tile([128, nb], BF, tag="agin")
                    nc.scalar.dma_start(out=ag_in[:], in_=o[:])
                    ag_out = dpool.tile([128 * ncores, nb], BF, tag="agout",
                                        addr_space="Shared")
                    nc.gpsimd.collective_compute(
                        "AllGather", mybir.AluOpType.bypass, replica_groups=rg,
                        ins=[ag_in[:].opt()], outs=[ag_out[:].opt()])
                    # scatter rank blocks into the next X slab (group m contiguous)
                    nc.sync.dma_start(
                        out=x_next[:, m],
                        in_=ag_out[:].rearrange("(r p) n -> p r n", p=128))

            for s in range(nsteps):
                last = (s == nsteps - 1)
                x_next = None if last else xpool.tile([128, MT, ncores, nb], BF, tag="x")
                if s == 0:
                    for m in range(MT):
                        psum = pspool.tile([128, nb], F32, tag="ps")
                        nc.tensor.matmul(psum[:], eye[:], xb_sb[:, m],
                                         start=True, stop=True)
                        epilogue(psum, m, s, x_next)
                else:
                    psums = [pspool.tile([128, nb], F32, name=f"ps_s{s}_m{m}", tag="ps")
                             for m in range(MT)]
                    ng = MT  # gather groups of previous step == MT
                    started = [False] * MT
                    # groups 0..ng-2 first for every m; defer group ng-1
                    for m in range(MT):
                        for g in range(ng - 1):
                            for r in range(ncores):
                                nc.tensor.matmul(psums[m][:], wT[:, r * MT + g,
                                                                m * 128:(m + 1) * 128],
                                                 x_cur[:, g, r],
                                                 start=not started[m], stop=False)
                                started[m] = True
                    for m in range(MT):
                        g = ng - 1
                        for r in range(ncores):
                            nc.tensor.matmul(psums[m][:], wT[:, r * MT + g,
                                                            m * 128:(m + 1) * 128],
                                             x_cur[:, g, r],
                                             start=not started[m], stop=False)
                            started[m] = True
                        nc.tensor.matmul(psums[m][:], eye[:], xb_sb[:, m],
                                         start=False, stop=True)
                        epilogue(psums[m], m, s, x_next)
                x_cur = x_next

    nc.compile()
    return nc


def _prep_in_maps(X_full, weights, bias, ncores):
    nn = weights.shape[0]
    R = nn // ncores
    XB = X_full.T.astype(np.float32) + bias.astype(np.float32)   # (nn, nb)
    eye = np.eye(128, dtype=np.float32)
    in_maps = []
    for c in range(ncores):
        Wc = weights[c * R:(c + 1) * R, :]
        in_maps.append({
            "wT": np.ascontiguousarray(Wc.T).astype(BF16NP),
            "xb": np.ascontiguousarray(XB[c * R:(c + 1) * R, :]),
            "eye": eye,
        })
    return in_maps


def kernel(X_full, weights, bias):
    nn = weights.shape[0]
    nb = X_full.shape[0]
    nc = build_nc(nn=nn, nb=nb, ncores=NCORES, nsteps=NSTEPS, debug=False)
    in_maps = _prep_in_maps(X_full, weights, bias, NCORES)
    res = run_bass_kernel_spmd(nc, in_maps, core_ids=list(range(NCORES)))
    blocks = [np.asarray(res.results[c]["out"], dtype=np.float32)
              for c in range(NCORES)]
    X_ss = np.concatenate(blocks, axis=0)          # (nn, nb)
    return np.ascontiguousarray(X_ss.T).astype(np.float32)
